# revision 24
# baseline (speedup 1.0000x reference)
import sys
import numpy as np

sys.path.insert(0, '/opt/trn_rl_repo')

import ml_dtypes
import concourse.bacc as bacc
import concourse.mybir as mybir
from concourse.bass_utils import run_bass_kernel_spmd
from concourse.tile import TileContext
from contextlib import ExitStack

f32 = mybir.dt.float32
f32r = mybir.dt.float32r
bf16 = mybir.dt.bfloat16
AF = mybir.ActivationFunctionType
ALU = mybir.AluOpType

D_MODEL = 1024
N_HEAD = 16
D_HEAD = 64
B = 4
T = 2048
N_CORES = 8
HPC = N_HEAD // 2        # 8 heads per core
HD = HPC * D_HEAD        # 512 head-dims per core
NTK = D_MODEL // 128     # 8 k-chunks over model dim
NTT = T // 128           # 16 T-tiles of 128

_cache = {}


def _build():
    nc = bacc.Bacc()
    xT = nc.declare_dram_parameter("xT", [D_MODEL, T], f32r, isOutput=False)
    # wqR rows m*128+p (p = in-dim within k-chunk), cols k*128+u (u = out-dim
    # within chunk m); m 0..3 = q (pre-scaled), 4..7 = k
    wqR = nc.declare_dram_parameter("wqR", [1024, 1024], f32r, isOutput=False)
    # wvR[p, k*512+c] = w_v[c, k*128+p]
    wvR = nc.declare_dram_parameter("wvR", [128, 8 * HD], f32r, isOutput=False)
    wpB = nc.declare_dram_parameter("wpB", [HD, D_MODEL], bf16, isOutput=False)
    maskB = nc.declare_dram_parameter("maskB", [128, 256], bf16, isOutput=False)
    onesB = nc.declare_dram_parameter("onesB", [128, 64], bf16, isOutput=False)
    outp = nc.declare_dram_parameter("out", [T, D_MODEL], f32, isOutput=True)

    with TileContext(nc) as tc, ExitStack() as ctx:
        sb = ctx.enter_context(tc.tile_pool(name="sb", bufs=1))
        pp = ctx.enter_context(tc.tile_pool(name="pp", bufs=1, space="PSUM"))

        # ---- constants, weights, x: spread over scalar + gpsimd queues
        # (the sync queue dribbles — it carries only output DMAs) ----
        mask = sb.tile([128, 256], bf16, tag="mask", name="mask")
        ones2 = sb.tile([128, 64], bf16, tag="ones2", name="ones2")
        nc.gpsimd.dma_start(out=mask[:], in_=maskB[:, :])
        nc.gpsimd.dma_start(out=ones2[:], in_=onesB[:, :])

        wq_tiles = {}

        def dma_wq(m):
            t_ = sb.tile([128, 1024], f32r, tag=f"wq{m}", name=f"wq{m}")
            nc.gpsimd.dma_start(out=t_[:], in_=wqR[m * 128:(m + 1) * 128, :])
            wq_tiles[m] = t_

        xt = {}

        def dma_x(th, j, ks, eng):
            c0 = th * 1024 + j * 512
            for k in ks:
                t_ = sb.tile([128, 512], f32r, tag=f"x{k}_{j}", bufs=2,
                             name=f"x{k}{j}{th}")
                eng.dma_start(out=t_[:], in_=xT[k * 128:(k + 1) * 128,
                                                c0:c0 + 512])
                xt[(k, th, j)] = t_

        dma_x(0, 0, range(4, 8), nc.gpsimd)
        dma_wq(0)
        dma_wq(4)
        dma_x(0, 0, range(0, 4), nc.scalar)
        wv = sb.tile([128, 8 * HD], f32r, tag="wv", name="wv")
        nc.scalar.dma_start(out=wv[:], in_=wvR[:, :])
        dma_x(0, 1, range(4, 8), nc.gpsimd)
        dma_x(0, 1, range(0, 4), nc.scalar)
        dma_wq(1)
        dma_wq(5)
        dma_x(1, 0, range(4, 8), nc.gpsimd)
        dma_x(1, 0, range(0, 4), nc.scalar)
        dma_x(1, 1, range(4, 8), nc.gpsimd)
        dma_x(1, 1, range(0, 4), nc.scalar)
        wp = []
        for mk in range(4):
            t_ = sb.tile([128, D_MODEL], bf16, tag=f"wp{mk}", name=f"wp{mk}")
            nc.gpsimd.dma_start(out=t_[:], in_=wpB[mk * 128:(mk + 1) * 128, :])
            wp.append(t_)
        for m in (2, 6, 3, 7):
            dma_wq(m)

        ysb = [sb.tile([128, T], bf16, tag=f"ysb{m}", name=f"ysb{m}")
               for m in range(4)]
        qk_tiles = {}

        def qk_tile(m):
            if m not in qk_tiles:
                qk_tiles[m] = sb.tile([128, T], f32r,
                                      tag=("qt" if m < 4 else "kt"), bufs=2,
                                      name=f"qk{m}")
            return qk_tiles[m]

        vt = {}

        # ---- static emission scheduler ----
        st = {"t": 0.0, "a": 0.0}

        _x_eta = {(0, 0): 8000.0, (0, 1): 16000.0, (1, 0): 26000.0,
                  (1, 1): 34000.0}

        def x_ready(th, j):
            return _x_eta[(th, j)]

        MMC = 512 / 2.4 + 15.0

        emitted = set()
        pending = {}   # half-emitted quantum: q -> psum tile

        def emit_s1a_half(m, th, j, h):
            st["t"] = max(st["t"], x_ready(th, j))
            q = ("a", m, th, j)
            if h == 0:
                ps_ = pp.tile([128, 512], f32, tag="fill", bufs=1, name="fillq")
                pending[q] = ps_
            else:
                ps_ = pending.pop(q)
                emitted.add(q)
            wqt = wq_tiles[m]
            for k in range(4 * h, 4 * h + 4):
                nc.tensor.matmul(ps_[:], wqt[:, k * 128:(k + 1) * 128],
                                 xt[(k, th, j)][:],
                                 start=(k == 0), stop=(k == NTK - 1))
            if h == 1:
                dst = qk_tile(m)
                c0 = th * 1024 + j * 512
                nc.vector.tensor_copy(dst[:, c0:c0 + 512], ps_[:])
            st["t"] += 4 * MMC

        def emit_s1b_half(th, tl, h):
            j, c = tl // 4, tl % 4
            st["t"] = max(st["t"], x_ready(th, j))
            q = ("b", th, tl)
            if h == 0:
                ps_ = pp.tile([128, 512], f32, tag="fill", bufs=1, name="fillv")
                pending[q] = ps_
            else:
                ps_ = pending.pop(q)
                emitted.add(q)
            for k in range(4 * h, 4 * h + 4):
                nc.tensor.matmul(ps_[:], xt[(k, th, j)][:, c * 128:(c + 1) * 128],
                                 wv[:, k * HD:(k + 1) * HD],
                                 start=(k == 0), stop=(k == NTK - 1))
            if h == 1:
                ti = 8 * th + tl
                v_ = sb.tile([128, HD], bf16, tag=f"v{ti}", name=f"v{ti}")
                nc.vector.tensor_copy(v_[:], ps_[:])
                vt[ti] = v_
            st["t"] += 4 * MMC

        def emit_half(q, h):
            if q[0] == "a":
                emit_s1a_half(q[1], q[2], q[3], h)
            else:
                emit_s1b_half(q[1], q[2], h)

        def emit_quantum(q):
            # both halves back-to-back (forced prereq path)
            if q in pending:
                emit_half(q, 1)
                return
            emit_half(q, 0)
            emit_half(q, 1)

        # global quanta order (wave = x block availability)
        quanta = []
        for J in range(4):
            th, j = J // 2, J % 2
            quanta.append(("a", 0, th, j))
            quanta.append(("a", 4, th, j))
            for c in range(4):
                quanta.append(("b", th, j * 4 + c))
            for m in (1, 5, 2, 6, 3, 7):
                quanta.append(("a", m, th, j))

        s4_avail = []
        s4_done = set()

        def emit_s4(tt, oc):
            s4_done.add((tt, oc))
            ps_ = pp.tile([128, 512], f32, tag="fill", bufs=1, name="fills4")
            for mk in range(4):
                nc.tensor.matmul(ps_[:], ysb[mk][:, tt * 128:(tt + 1) * 128],
                                 wp[mk][:, oc * 512:(oc + 1) * 512],
                                 start=(mk == 0), stop=(mk == 3))
            o_ = sb.tile([128, 512], f32, tag="ob", bufs=2, name="obt")
            nc.vector.tensor_copy(o_[:], ps_[:])
            nc.gpsimd.dma_start(out=outp[tt * 128:(tt + 1) * 128,
                                         oc * 512:(oc + 1) * 512], in_=o_[:])
            st["t"] += 4 * MMC

        phase = {"p": 0}

        def next_filler():
            # a pending half-quantum must be finished before a new fill
            # tile can be allocated (fill tag has a single buffer)
            if pending:
                return ("h1", next(iter(pending)))
            p = phase["p"]
            win = {p, 4 + p}
            if p < 3:
                win |= {p + 1, 5 + p}
            for q in quanta:
                if q in emitted:
                    continue
                if q[0] == "a":
                    if q[1] not in win:
                        continue
                    rt = x_ready(q[2], q[3])
                else:
                    rt = x_ready(q[1], q[2] // 4)
                if rt <= st["t"] + 500:
                    return ("h0", q)
            if s4_avail:
                return ("s4", s4_avail.pop(0))
            return None

        def fill_until(target):
            while st["t"] < target - 300:
                f = next_filler()
                if f is None:
                    break
                if f[0] == "h0":
                    emit_half(f[1], 0)
                elif f[0] == "h1":
                    emit_half(f[1], 1)
                else:
                    emit_s4(*f[1])
            st["t"] = max(st["t"], target)

        def force_prereqs(need):
            for q in need:
                if q not in emitted:
                    emit_quantum(q)

        # ---- attention group ----
        def attention_group(p, J):
            qt, kt = qk_tile(p), qk_tile(4 + p)
            iN = 4 * J + 4
            q0 = 512 * J
            psy = pp.tile([128, 512], f32, tag="psy", bufs=2, name="psyt")
            den = pp.tile([128, 512], f32, tag="den", bufs=1, name="dent")
            pts = {}
            exp_end = {}

            def qk_unit(i):
                su = max(0, 128 * i - q0)
                su2 = min(su, 256)
                n = 512 - su2
                psa = pp.tile([128, 1024], f32, tag="psa", bufs=2, name="psat")
                nc.tensor.matmul(psa[:, su2:512],
                                 kt[0:64, i * 128:(i + 1) * 128],
                                 qt[0:64, q0 + su2:q0 + 512],
                                 start=True, stop=True, tile_position=(0, 0))
                nc.tensor.matmul(psa[:, 512 + su2:1024],
                                 kt[64:128, i * 128:(i + 1) * 128],
                                 qt[64:128, q0 + su2:q0 + 512],
                                 start=True, stop=True, tile_position=(64, 0))
                st["t"] += n / 2.4 + 40
                pt = sb.tile([128, 1024], bf16, tag="pt", bufs=4, name="ptt")
                p3i = psa[:].rearrange("p (g c) -> p g c", g=2)
                p3o = pt[:].rearrange("p (g c) -> p g c", g=2)
                nc.scalar.activation(p3o[:, :, su2:512], p3i[:, :, su2:512],
                                     AF.Exp)
                es = max(st["a"], st["t"] + 250)
                st["a"] = es + (2 * n + 352) / 1.2
                exp_end[i] = st["a"]
                if su > su2:
                    nc.vector.memset(p3o[:, :, su2:su], 0)
                if i >= 4 * J:
                    mask3 = mask[:].rearrange("p (g c) -> p g c", g=2)
                    nc.vector.tensor_tensor(p3o[:, :, su:su + 128],
                                            p3o[:, :, su:su + 128],
                                            mask3[:, :, :], ALU.mult)
                    exp_end[i] += 250
                pts[i] = (pt, su2)

            def pv_unit(i):
                pt, su2 = pts.pop(i)
                n = 512 - su2
                first, last = (i == 0), (i == iN - 1)
                vA = vt[i][:, 128 * p:128 * p + 64]
                vB = vt[i][:, 128 * p + 64:128 * p + 128]
                nc.tensor.matmul(psy[0:64, su2:512], vA, pt[:, su2:512],
                                 start=first, stop=last, tile_position=(0, 0))
                nc.tensor.matmul(psy[64:128, su2:512], vB,
                                 pt[:, 512 + su2:1024],
                                 start=first, stop=last, tile_position=(0, 64))
                nc.tensor.matmul(den[0:64, su2:512], ones2[:, :],
                                 pt[:, su2:512],
                                 start=first, stop=last, tile_position=(0, 0))
                nc.tensor.matmul(den[64:128, su2:512], ones2[:, :],
                                 pt[:, 512 + su2:1024],
                                 start=first, stop=last,
                                 tile_position=(0, 64))
                st["t"] += 2 * (n / 2.4) + 60

            for i in range(iN):
                if i >= 2:
                    fill_until(exp_end[i - 2] + 250)
                qk_unit(i)
                if i >= 1:
                    fill_until(exp_end[i - 1] + 250)
                    pv_unit(i - 1)
            fill_until(exp_end[iN - 1] + 250)
            pv_unit(iN - 1)

            # normalize: den rows are already replicated across partitions
            # (ones lhsT is [128,64]); fast approx 1/den, then multiply.
            # Single full-tile ops — the custom DVE op mishandles APs with
            # a nonzero partition base.
            bc = sb.tile([128, 512], f32, tag="bc", bufs=2, name="bc")
            nc.vector.reciprocal_approx_fast(bc[:], den[:, :])
            nc.vector.tensor_tensor(ysb[p][:, q0:q0 + 512], psy[:, :],
                                    bc[:], ALU.mult)

        # ---- main emission loop: m-major, J-inner ----
        for p in range(4):
            phase["p"] = p
            for J in range(4):
                th, j = J // 2, J % 2
                need = [("a", p, th, j), ("a", 4 + p, th, j)]
                need += [("b", th, j * 4 + c) for c in range(4)]
                force_prereqs(need)
                attention_group(p, J)
                if p == 3:
                    for tt in range(4 * J, 4 * J + 4):
                        s4_avail.append((tt, 0))
                        s4_avail.append((tt, 1))

        # ---- tail: remaining S4 tiles ----
        for tt in range(NTT):
            for oc in range(2):
                if (tt, oc) not in s4_done:
                    emit_s4(tt, oc)

    nc.compile()
    return nc


def _prep_core_inputs(x, w_qkv, w_proj, c):
    b, g = c // 2, c % 2
    scale = np.float32(D_HEAD ** -0.5)
    wq = (w_qkv[g * HD:(g + 1) * HD] * scale).astype(np.float32)
    wk = w_qkv[D_MODEL + g * HD:D_MODEL + (g + 1) * HD]
    wv = w_qkv[2 * D_MODEL + g * HD:2 * D_MODEL + (g + 1) * HD]
    A = np.concatenate([wq, wk], 0)                     # [1024 out, 1024 in]
    wqR = A.reshape(8, 128, 8, 128).transpose(0, 3, 2, 1).reshape(1024, 1024)
    wvR = wv.reshape(512, 8, 128).transpose(2, 1, 0).reshape(128, 8 * 512)
    tri = np.triu(np.ones((128, 128), dtype=np.float32))
    return {
        "xT": np.ascontiguousarray(x[b].T),
        "wqR": np.ascontiguousarray(wqR),
        "wvR": np.ascontiguousarray(wvR),
        "wpB": np.ascontiguousarray(
            w_proj[:, g * HD:(g + 1) * HD].T).astype(ml_dtypes.bfloat16),
        "maskB": np.concatenate([tri, tri], 1).astype(ml_dtypes.bfloat16),
        "onesB": np.ones((128, 64), dtype=ml_dtypes.bfloat16),
    }


def kernel(x, w_qkv, w_proj):
    x = np.asarray(x)
    w_qkv = np.asarray(w_qkv)
    w_proj = np.asarray(w_proj)
    if "nc" not in _cache:
        _cache["nc"] = _build()
    nc = _cache["nc"]
    in_maps = [_prep_core_inputs(x, w_qkv, w_proj, c) for c in range(N_CORES)]
    res = run_bass_kernel_spmd(nc, in_maps, core_ids=list(range(N_CORES)))
    outs = [res.results[c]["out"] for c in range(N_CORES)]
    return np.stack([outs[2 * b] + outs[2 * b + 1] for b in range(B)], 0)


# revision 28
# speedup vs baseline: 1.2526x; 1.2526x over previous
import sys
import numpy as np

sys.path.insert(0, '/opt/trn_rl_repo')

import ml_dtypes
import concourse.bacc as bacc
import concourse.mybir as mybir
from concourse.bass_utils import run_bass_kernel_spmd
from concourse.tile import TileContext
from contextlib import ExitStack

f32 = mybir.dt.float32
f32r = mybir.dt.float32r
bf16 = mybir.dt.bfloat16
AF = mybir.ActivationFunctionType
ALU = mybir.AluOpType

D_MODEL = 1024
N_HEAD = 16
D_HEAD = 64
B = 4
T = 2048
N_CORES = 8
HPC = N_HEAD // 2        # 8 heads per core
HD = HPC * D_HEAD        # 512 head-dims per core
NTK = D_MODEL // 128     # 8 k-chunks over model dim
NTT = T // 128           # 16 T-tiles of 128

_cache = {}


def _build():
    nc = bacc.Bacc()
    xT = nc.declare_dram_parameter("xT", [D_MODEL, T], f32r, isOutput=False)
    # wqR rows m*128+p (p = in-dim within k-chunk), cols k*128+u (u = out-dim
    # within chunk m); m 0..3 = q (pre-scaled), 4..7 = k
    wqR = nc.declare_dram_parameter("wqR", [1024, 1024], f32r, isOutput=False)
    # wvR[p, k*512+c] = w_v[c, k*128+p]
    wvR = nc.declare_dram_parameter("wvR", [128, 8 * HD], f32r, isOutput=False)
    wpB = nc.declare_dram_parameter("wpB", [HD, D_MODEL], bf16, isOutput=False)
    maskB = nc.declare_dram_parameter("maskB", [128, 256], f32, isOutput=False)
    onesB = nc.declare_dram_parameter("onesB", [128, 64], bf16, isOutput=False)
    outp = nc.declare_dram_parameter("out", [T, D_MODEL], f32, isOutput=True)

    with TileContext(nc) as tc, ExitStack() as ctx:
        sb = ctx.enter_context(tc.tile_pool(name="sb", bufs=1))
        pp = ctx.enter_context(tc.tile_pool(name="pp", bufs=1, space="PSUM"))

        # ---- constants, weights, x: spread over scalar + gpsimd queues
        # (the sync queue dribbles — it carries only output DMAs) ----
        mask = sb.tile([128, 256], f32, tag="mask", name="mask")
        ones2 = sb.tile([128, 64], bf16, tag="ones2", name="ones2")
        nc.gpsimd.dma_start(out=mask[:], in_=maskB[:, :])
        nc.gpsimd.dma_start(out=ones2[:], in_=onesB[:, :])

        wq_tiles = {}

        def dma_wq(m):
            t_ = sb.tile([128, 1024], f32r, tag=f"wq{m}", name=f"wq{m}")
            nc.gpsimd.dma_start(out=t_[:], in_=wqR[m * 128:(m + 1) * 128, :])
            wq_tiles[m] = t_

        xt = {}

        def dma_x(th, j, ks, eng):
            c0 = th * 1024 + j * 512
            for k in ks:
                t_ = sb.tile([128, 512], f32r, tag=f"x{k}_{j}", bufs=2,
                             name=f"x{k}{j}{th}")
                eng.dma_start(out=t_[:], in_=xT[k * 128:(k + 1) * 128,
                                                c0:c0 + 512])
                xt[(k, th, j)] = t_

        dma_x(0, 0, range(4, 8), nc.gpsimd)
        dma_wq(0)
        dma_wq(4)
        dma_x(0, 0, range(0, 4), nc.scalar)
        wv = sb.tile([128, 8 * HD], f32r, tag="wv", name="wv")
        nc.scalar.dma_start(out=wv[:], in_=wvR[:, :])
        dma_x(0, 1, range(4, 8), nc.gpsimd)
        dma_x(0, 1, range(0, 4), nc.scalar)
        dma_wq(1)
        dma_wq(5)
        dma_x(1, 0, range(4, 8), nc.gpsimd)
        dma_x(1, 0, range(0, 4), nc.scalar)
        dma_x(1, 1, range(4, 8), nc.gpsimd)
        dma_x(1, 1, range(0, 4), nc.scalar)
        wp = []
        for mk in range(4):
            t_ = sb.tile([128, D_MODEL], bf16, tag=f"wp{mk}", name=f"wp{mk}")
            nc.gpsimd.dma_start(out=t_[:], in_=wpB[mk * 128:(mk + 1) * 128, :])
            wp.append(t_)
        for m in (2, 6, 3, 7):
            dma_wq(m)

        ysb = [sb.tile([128, T], bf16, tag=f"ysb{m}", name=f"ysb{m}")
               for m in range(4)]
        qk_tiles = {}

        def qk_tile(m):
            if m not in qk_tiles:
                qk_tiles[m] = sb.tile([128, T], f32r,
                                      tag=("qt" if m < 4 else "kt"), bufs=2,
                                      name=f"qk{m}")
            return qk_tiles[m]

        vt = {}

        # ---- static emission scheduler ----
        st = {"t": 0.0, "a": 0.0}

        _x_eta = {(0, 0): 8000.0, (0, 1): 16000.0, (1, 0): 26000.0,
                  (1, 1): 34000.0}

        def x_ready(th, j):
            return _x_eta[(th, j)]

        MMC = 512 / 2.4 + 15.0

        emitted = set()
        pending = {}   # half-emitted quantum: q -> psum tile

        def emit_s1a_half(m, th, j, h):
            st["t"] = max(st["t"], x_ready(th, j))
            q = ("a", m, th, j)
            if h == 0:
                ps_ = pp.tile([128, 512], f32, tag="fill", bufs=2, name="fillq")
                pending[q] = ps_
            else:
                ps_ = pending.pop(q)
                emitted.add(q)
            wqt = wq_tiles[m]
            for k in range(4 * h, 4 * h + 4):
                nc.tensor.matmul(ps_[:], wqt[:, k * 128:(k + 1) * 128],
                                 xt[(k, th, j)][:],
                                 start=(k == 0), stop=(k == NTK - 1))
            if h == 1:
                dst = qk_tile(m)
                c0 = th * 1024 + j * 512
                nc.vector.tensor_copy(dst[:, c0:c0 + 512], ps_[:])
            st["t"] += 4 * MMC

        def emit_s1b_half(th, tl, h):
            j, c = tl // 4, tl % 4
            st["t"] = max(st["t"], x_ready(th, j))
            q = ("b", th, tl)
            if h == 0:
                ps_ = pp.tile([128, 512], f32, tag="fill", bufs=2, name="fillv")
                pending[q] = ps_
            else:
                ps_ = pending.pop(q)
                emitted.add(q)
            for k in range(4 * h, 4 * h + 4):
                nc.tensor.matmul(ps_[:], xt[(k, th, j)][:, c * 128:(c + 1) * 128],
                                 wv[:, k * HD:(k + 1) * HD],
                                 start=(k == 0), stop=(k == NTK - 1))
            if h == 1:
                ti = 8 * th + tl
                v_ = sb.tile([128, HD], bf16, tag=f"v{ti}", name=f"v{ti}")
                nc.vector.tensor_copy(v_[:], ps_[:])
                vt[ti] = v_
            st["t"] += 4 * MMC

        def emit_half(q, h):
            if q[0] == "a":
                emit_s1a_half(q[1], q[2], q[3], h)
            else:
                emit_s1b_half(q[1], q[2], h)

        def emit_quantum(q):
            # both halves back-to-back (forced prereq path)
            if q in pending:
                emit_half(q, 1)
                return
            emit_half(q, 0)
            emit_half(q, 1)

        # global quanta order (wave = x block availability)
        quanta = []
        for J in range(4):
            th, j = J // 2, J % 2
            quanta.append(("a", 0, th, j))
            quanta.append(("a", 4, th, j))
            for c in range(4):
                quanta.append(("b", th, j * 4 + c))
            for m in (1, 5, 2, 6, 3, 7):
                quanta.append(("a", m, th, j))

        s4_avail = []
        s4_done = set()

        def emit_s4(tt, oc):
            s4_done.add((tt, oc))
            ps_ = pp.tile([128, 512], f32, tag="fill", bufs=2, name="fills4")
            for mk in range(4):
                nc.tensor.matmul(ps_[:], ysb[mk][:, tt * 128:(tt + 1) * 128],
                                 wp[mk][:, oc * 512:(oc + 1) * 512],
                                 start=(mk == 0), stop=(mk == 3))
            o_ = sb.tile([128, 512], f32, tag="ob", bufs=2, name="obt")
            nc.vector.tensor_copy(o_[:], ps_[:])
            nc.gpsimd.dma_start(out=outp[tt * 128:(tt + 1) * 128,
                                         oc * 512:(oc + 1) * 512], in_=o_[:])
            st["t"] += 4 * MMC

        phase = {"p": 0}

        def next_filler():
            # a pending half-quantum must be finished before a new fill
            # tile can be allocated (fill tag has a single buffer)
            if pending:
                return ("h1", next(iter(pending)))
            p = phase["p"]
            win = {p, 4 + p}
            if p < 3:
                win |= {p + 1, 5 + p}
            for q in quanta:
                if q in emitted:
                    continue
                if q[0] == "a":
                    if q[1] not in win:
                        continue
                    rt = x_ready(q[2], q[3])
                else:
                    rt = x_ready(q[1], q[2] // 4)
                if rt <= st["t"] + 500:
                    return ("h0", q)
            if s4_avail:
                return ("s4", s4_avail.pop(0))
            return None

        def fill_until(target):
            while st["t"] < target - 300:
                f = next_filler()
                if f is None:
                    break
                if f[0] == "h0":
                    emit_half(f[1], 0)
                elif f[0] == "h1":
                    emit_half(f[1], 1)
                else:
                    emit_s4(*f[1])
            st["t"] = max(st["t"], target)

        def force_prereqs(need):
            for q in need:
                if q not in emitted:
                    emit_quantum(q)

        # ---- attention group ----
        def attention_group(p, J):
            qt, kt = qk_tile(p), qk_tile(4 + p)
            iN = 4 * J + 4
            q0 = 512 * J
            psy = pp.tile([128, 512], f32, tag="psy", bufs=1, name="psyt")
            den = pp.tile([128, 512], f32, tag="den", bufs=1, name="dent")
            pts = {}
            exp_end = {}

            def qk_unit(i):
                su = max(0, 128 * i - q0)
                su2 = min(su, 256)
                n = 512 - su2
                psa = pp.tile([128, 1024], f32, tag="psa", bufs=2, name="psat")
                nc.tensor.matmul(psa[:, su2:512],
                                 kt[0:64, i * 128:(i + 1) * 128],
                                 qt[0:64, q0 + su2:q0 + 512],
                                 start=True, stop=True, tile_position=(0, 0))
                nc.tensor.matmul(psa[:, 512 + su2:1024],
                                 kt[64:128, i * 128:(i + 1) * 128],
                                 qt[64:128, q0 + su2:q0 + 512],
                                 start=True, stop=True, tile_position=(64, 0))
                st["t"] += n / 2.4 + 40
                pt = sb.tile([128, 1024], bf16, tag="pt", bufs=4, name="ptt")
                p3i = psa[:].rearrange("p (g c) -> p g c", g=2)
                p3o = pt[:].rearrange("p (g c) -> p g c", g=2)
                if i >= 4 * J:
                    # additive -inf-style causal mask before the exp (keeps
                    # the exp -> PV dependency chain a single hop)
                    mask3 = mask[:].rearrange("p (g c) -> p g c", g=2)
                    nc.vector.tensor_tensor(p3i[:, :, su:su + 128],
                                            p3i[:, :, su:su + 128],
                                            mask3[:, :, :], ALU.add)
                nc.scalar.activation(p3o[:, :, su2:512], p3i[:, :, su2:512],
                                     AF.Exp)
                es = max(st["a"], st["t"] + 250)
                st["a"] = es + (2 * n + 352) / 1.2
                exp_end[i] = st["a"]
                if su > su2:
                    nc.vector.memset(p3o[:, :, su2:su], 0)
                pts[i] = (pt, su2)

            def pv_unit(i):
                pt, su2 = pts.pop(i)
                n = 512 - su2
                first, last = (i == 0), (i == iN - 1)
                vA = vt[i][:, 128 * p:128 * p + 64]
                vB = vt[i][:, 128 * p + 64:128 * p + 128]
                nc.tensor.matmul(psy[0:64, su2:512], vA, pt[:, su2:512],
                                 start=first, stop=last, tile_position=(0, 0))
                nc.tensor.matmul(psy[64:128, su2:512], vB,
                                 pt[:, 512 + su2:1024],
                                 start=first, stop=last, tile_position=(0, 64))
                nc.tensor.matmul(den[0:64, su2:512], ones2[:, :],
                                 pt[:, su2:512],
                                 start=first, stop=last, tile_position=(0, 0))
                nc.tensor.matmul(den[64:128, su2:512], ones2[:, :],
                                 pt[:, 512 + su2:1024],
                                 start=first, stop=last,
                                 tile_position=(0, 64))
                st["t"] += 2 * (n / 2.4) + 60

            for i in range(iN):
                if i >= 2:
                    fill_until(exp_end[i - 2] + 400)
                qk_unit(i)
                if i >= 1:
                    fill_until(exp_end[i - 1] + 400)
                    pv_unit(i - 1)
            fill_until(exp_end[iN - 1] + 400)
            pv_unit(iN - 1)

            # normalize: den rows are already replicated across partitions
            # (ones lhsT is [128,64]); fast approx 1/den, then multiply.
            # Single full-tile ops — the custom DVE op mishandles APs with
            # a nonzero partition base.
            bc = sb.tile([128, 512], f32, tag="bc", bufs=2, name="bc")
            nc.vector.reciprocal_approx_fast(bc[:], den[:, :])
            nc.vector.tensor_tensor(ysb[p][:, q0:q0 + 512], psy[:, :],
                                    bc[:], ALU.mult)

        # ---- main emission loop: m-major, J-inner ----
        for p in range(4):
            phase["p"] = p
            for J in range(4):
                th, j = J // 2, J % 2
                need = [("a", p, th, j), ("a", 4 + p, th, j)]
                need += [("b", th, j * 4 + c) for c in range(4)]
                force_prereqs(need)
                attention_group(p, J)
                if p == 3:
                    for tt in range(4 * J, 4 * J + 4):
                        s4_avail.append((tt, 0))
                        s4_avail.append((tt, 1))

        # ---- tail: remaining S4 tiles ----
        for tt in range(NTT):
            for oc in range(2):
                if (tt, oc) not in s4_done:
                    emit_s4(tt, oc)

    nc.compile()
    return nc


def _prep_core_inputs(x, w_qkv, w_proj, c):
    b, g = c // 2, c % 2
    scale = np.float32(D_HEAD ** -0.5)
    wq = (w_qkv[g * HD:(g + 1) * HD] * scale).astype(np.float32)
    wk = w_qkv[D_MODEL + g * HD:D_MODEL + (g + 1) * HD]
    wv = w_qkv[2 * D_MODEL + g * HD:2 * D_MODEL + (g + 1) * HD]
    A = np.concatenate([wq, wk], 0)                     # [1024 out, 1024 in]
    wqR = A.reshape(8, 128, 8, 128).transpose(0, 3, 2, 1).reshape(1024, 1024)
    wvR = wv.reshape(512, 8, 128).transpose(2, 1, 0).reshape(128, 8 * 512)
    tri = np.triu(np.ones((128, 128), dtype=np.float32))
    maskn = ((1.0 - tri) * np.float32(-1e30)).astype(np.float32)
    return {
        "xT": np.ascontiguousarray(x[b].T),
        "wqR": np.ascontiguousarray(wqR),
        "wvR": np.ascontiguousarray(wvR),
        "wpB": np.ascontiguousarray(
            w_proj[:, g * HD:(g + 1) * HD].T).astype(ml_dtypes.bfloat16),
        "maskB": np.concatenate([maskn, maskn], 1),
        "onesB": np.ones((128, 64), dtype=ml_dtypes.bfloat16),
    }


def kernel(x, w_qkv, w_proj):
    x = np.asarray(x)
    w_qkv = np.asarray(w_qkv)
    w_proj = np.asarray(w_proj)
    if "nc" not in _cache:
        _cache["nc"] = _build()
    nc = _cache["nc"]
    in_maps = [_prep_core_inputs(x, w_qkv, w_proj, c) for c in range(N_CORES)]
    res = run_bass_kernel_spmd(nc, in_maps, core_ids=list(range(N_CORES)))
    outs = [res.results[c]["out"] for c in range(N_CORES)]
    return np.stack([outs[2 * b] + outs[2 * b + 1] for b in range(B)], 0)
